# revision 26
# baseline (speedup 1.0000x reference)
"""AnomalyMapGenerator (retrieval kNN) Trainium2 kernel.

reference:  d = sqrt(distance[B, HW, M]); v = 3 smallest of d per row;
            w = softmax(-v); s = w0*v0 -> [B, 56, 56]
            -> bilinear resize to 224x224 -> gaussian blur (sigma=4, reflect).

Strategy (8 NeuronCores, data-parallel over batch, 2 images per core):
  - per core, rows r = b*3136 + hw (6272 rows of 4096 distances).
    Row->SBUF mapping r = 49*p + t: tile t holds rows {49p+t}, so the
    per-row scalar s lands in SBUF as [128, 49] in linear row order.
  - main loop (49 tiles of [128, 4096] f32, 2 MiB each):
      DMA load -> ScalarE negate -> VectorE max8 (top-8 of -d = 3 smallest of d,
      duplicate multiplicity preserved, matching lax.top_k).
  - tail: v = sqrt(-top3) (ScalarE), E = exp(-v) (ScalarE),
      s = v0*E0 / (E0+E1+E2) (VectorE), DMA s -> DRAM smap [6272].
  - post: resize+blur are one linear operator A = G_blur @ R_resize [224, 56];
      per image: out = A @ S @ A^T via two TensorE matmuls with
      amat_t = A^T [56, 224] (both stages use the same operand, no transposes).
"""
import os
import numpy as np

B, HW, M = 16, 3136, 4096
IMG_IN, IMG_OUT, SIGMA = 56, 224, 4.0
N_CORES = 8
BPC = B // N_CORES            # images per core
ROWS = BPC * HW               # 6272
P = 128
T = ROWS // P                 # 49 columns, row r = 49p + t
HALF = IMG_OUT // 2           # 112

# SDMA engine n of a transfer handles the n-th contiguous chunk of
# ceil(D/16) descriptors (measured).  Engine 15 is intermittently ~15-20%
# slower than the rest, so a few load units are issued as a [0:120)
# transfer (engines 0-14 only: 120 descriptors = 15 chunks of 8) plus a
# [120:128) transfer (8 descriptors -> engines 0-7), shifting ~16% of
# engine 15's bytes onto the others.
SPLIT_UNITS = {3, 7}

_CACHE = {}


def _resize_mat(in_size: int, out_size: int) -> np.ndarray:
    # jax.image.resize(method='bilinear') upsampling weight matrix [out, in]
    scale = out_size / in_size
    sample_f = (np.arange(out_size, dtype=np.float64) + 0.5) / scale - 0.5
    x = np.abs(sample_f[None, :] - np.arange(in_size, dtype=np.float64)[:, None])
    w = np.maximum(0.0, 1.0 - x)
    total = w.sum(axis=0, keepdims=True)
    w = np.where(np.abs(total) > 1e-8, w / total, 0.0)
    ob = (sample_f < -0.5) | (sample_f > in_size - 0.5)
    w[:, ob] = 0.0
    return w.T


def _gauss_mat(n: int, sigma: float) -> np.ndarray:
    # 1D gaussian conv with reflect padding as a matrix [n, n]
    ksize = 2 * int(4.0 * sigma + 0.5) + 1
    xs = np.arange(ksize, dtype=np.float64) - ksize // 2
    g = np.exp(-(xs * xs) / (2.0 * sigma * sigma))
    g = g / g.sum()
    pad = ksize // 2
    Gm = np.zeros((n, n), dtype=np.float64)
    for o in range(n):
        for k in range(ksize):
            idx = o - pad + k
            if idx < 0:
                idx = -idx
            elif idx > n - 1:
                idx = 2 * (n - 1) - idx
            Gm[o, idx] += g[k]
    return Gm


def _amat_t() -> np.ndarray:
    A = _gauss_mat(IMG_OUT, SIGMA) @ _resize_mat(IMG_IN, IMG_OUT)  # [224, 56]
    return np.ascontiguousarray(A.T.astype(np.float32))            # [56, 224]


def _build():
    from contextlib import ExitStack
    import concourse.bass as bass
    import concourse.tile as tile
    from concourse import bacc, mybir

    f32 = mybir.dt.float32
    AF = mybir.ActivationFunctionType

    nc = bacc.Bacc("TRN2", target_bir_lowering=False, debug=False,
                   enable_asserts=False)
    dist = nc.dram_tensor("distance", [ROWS, M], f32, kind="ExternalInput")
    amat = nc.dram_tensor("amat_t", [IMG_IN, IMG_OUT], f32, kind="ExternalInput")
    out = nc.dram_tensor("out", [BPC, IMG_OUT, IMG_OUT], f32, kind="ExternalOutput")
    smap = nc.dram_tensor("smap", [ROWS], f32)  # internal scratch

    distv = dist.ap().rearrange("(p t) m -> p t m", p=P)      # r = 49p + t
    smap_pt = smap.ap().rearrange("(p t) -> p t", p=P)
    smap_img = smap.ap().rearrange("(i h w) -> i h w", i=BPC, h=IMG_IN)
    out_ap = out.ap()

    with tile.TileContext(nc) as tc, ExitStack() as ctx:
        pool_in = ctx.enter_context(tc.tile_pool(name="in", bufs=2))
        pool_neg = ctx.enter_context(tc.tile_pool(name="neg", bufs=3))
        pool_keep = ctx.enter_context(tc.tile_pool(name="keep", bufs=1))
        pool_mm = ctx.enter_context(tc.tile_pool(name="mm", bufs=2))
        pool_ps = ctx.enter_context(
            tc.tile_pool(name="ps", bufs=4, space="PSUM"))

        # preload the sqrt activation table before ScalarE gets busy so the
        # tail pays no ACT_TABLE_LOAD (the in-loop negates are Copy, which is
        # in every table set, so sqrt stays resident)
        warm = pool_keep.tile([P, 8], f32)
        nc.vector.memset(warm[:], 1.0)
        nc.scalar.activation(warm[:], warm[:], AF.Sqrt)

        top8 = pool_keep.tile([P, 8 * T], f32)
        # quad loads: 4 adjacent t-columns are 64 KiB contiguous/partition
        # (bigger descriptors amortize the ~85 ns/packet fixed cost).
        # negate per COLUMN so max8s start before the whole unit is negated;
        # the stream end tapers to pairs/singles to keep the drain short.
        units = ([(4 * j, 4) for j in range(10)]
                 + [(40, 2), (42, 2), (44, 1), (45, 1)])
        for u, (t0, w) in enumerate(units):
            tin = pool_in.tile([P, w * M], f32, tag="in")
            dst = tin[:].rearrange("p (k m) -> p k m", k=w)
            src = distv[:, t0:t0 + w, :]
            if u in SPLIT_UNITS:
                nc.sync.dma_start(dst[0:120], src[0:120])
                nc.sync.dma_start(dst[120:P], src[120:P])
            else:
                nc.sync.dma_start(dst, src)
            for k in range(w):
                t = t0 + k
                tneg = pool_neg.tile([P, M], f32, tag="neg")
                nc.scalar.mul(tneg[:], tin[:, k * M:(k + 1) * M], -1.0)
                nc.vector.max(top8[:, 8 * t:8 * t + 8], tneg[:])
        # stream-end taper: chunked columns so the post-DMA drain chain is
        # short; top-8 of a row = top-8 of the merged per-chunk top-8s
        for t, n_chunks in ((46, 2), (47, 2), (48, 4)):
            cw = M // n_chunks
            parts16 = pool_keep.tile([P, 8 * n_chunks], f32,
                                     tag=f"parts{n_chunks}")
            for h in range(n_chunks):
                tin = pool_in.tile([P, cw], f32, tag="in")
                nc.sync.dma_start(tin[:], distv[:, t, h * cw:(h + 1) * cw])
                tneg = pool_neg.tile([P, cw], f32, tag="neg")
                nc.scalar.mul(tneg[:], tin[:], -1.0)
                nc.vector.max(parts16[:, 8 * h:8 * h + 8], tneg[:])
            nc.vector.max(top8[:, 8 * t:8 * t + 8], parts16[:])

        amat_sb = pool_keep.tile([IMG_IN, IMG_OUT], f32)
        nc.sync.dma_start(amat_sb[:], amat.ap())

        # tail: softmin-weighted minimum per row
        #   s = v0 / (1 + e^{d1} + e^{d2}),  d_j = v0 - v_j  in [-1, 0]
        # One Sqrt activation (table warm); the exponentials use a cubic
        # Taylor poly on VectorE -- d is the gap between the closest and
        # 2nd/3rd-closest of 4096 distances, |d| <~ 0.05, so the cubic is
        # exact to ~1e-8 (worst-case |d|=1 error 0.035 still << tolerance).
        top8v = top8[:].rearrange("p (t e) -> p e t", e=8)
        vall = pool_keep.tile([P, 3 * T], f32)   # [v0 | v1 | v2] blocks
        nc.scalar.activation(
            vall[:].rearrange("p (e t) -> p e t", e=3),
            top8v[:, 0:3, :], AF.Sqrt, scale=-1.0)
        v0, v1, v2 = (vall[:, e * T:(e + 1) * T] for e in range(3))
        dd = pool_keep.tile([P, 2 * T], f32)
        nc.vector.tensor_sub(dd[:, 0:T], v0, v1)
        nc.vector.tensor_sub(dd[:, T:2 * T], v0, v2)
        # Horner: e^d ~ ((d/6 + 1/2)d + 1)d + 1
        ee = pool_keep.tile([P, 2 * T], f32)
        nc.vector.tensor_scalar(ee[:], dd[:], 1.0 / 6.0, 0.5,
                                mybir.AluOpType.mult, mybir.AluOpType.add)
        nc.vector.tensor_mul(ee[:], ee[:], dd[:])
        nc.vector.tensor_scalar_add(ee[:], ee[:], 1.0)
        nc.vector.tensor_mul(ee[:], ee[:], dd[:])
        denom = pool_keep.tile([P, T], f32)
        nc.vector.tensor_add(denom[:], ee[:, 0:T], ee[:, T:2 * T])
        # denom currently holds (e^{d1}-1) + (e^{d2}-1); add the 3 ones
        nc.vector.tensor_scalar_add(denom[:], denom[:], 3.0)
        rec = pool_keep.tile([P, T], f32)
        nc.vector.reciprocal(rec[:], denom[:])
        sval = pool_keep.tile([P, T], f32)
        nc.vector.tensor_mul(sval[:], v0, rec[:])
        # per-image halves: partitions [0,64) hold image 0's rows;
        # interleave each half's write with its S re-load
        s_tiles = []
        for i in range(BPC):
            h0, h1 = i * (P // 2), (i + 1) * (P // 2)
            nc.sync.dma_start(smap_pt[h0:h1, :], sval[h0:h1, :])
            s_i = pool_mm.tile([IMG_IN, IMG_IN], f32)
            nc.sync.dma_start(s_i[:], smap_img[i, :, :])
            s_tiles.append(s_i)

        # post: out_i = A @ S_i @ A^T
        for i in range(BPC):
            s_i = s_tiles[i]
            ps1 = pool_ps.tile([IMG_IN, IMG_OUT], f32)
            # ps1[w', n] = sum_h S[h, w'] * A[n, h]  ==  (A @ S)^T
            nc.tensor.matmul(ps1[:], s_i[:], amat_sb[:], start=True, stop=True)
            u1 = pool_mm.tile([IMG_IN, IMG_OUT], f32)
            (nc.vector.tensor_copy if i else nc.scalar.copy)(u1[:], ps1[:])
            o_all = pool_mm.tile([HALF, 2 * IMG_OUT], f32)
            for c in range(2):
                ps2 = pool_ps.tile([HALF, IMG_OUT], f32)
                # ps2[ho, w] = sum_w' (A@S)[ho, w'] * A[w, w']
                nc.tensor.matmul(ps2[:], u1[:, c * HALF:(c + 1) * HALF],
                                 amat_sb[:], start=True, stop=True)
                (nc.vector.tensor_copy if c else nc.scalar.copy)(
                    o_all[:, c * IMG_OUT:(c + 1) * IMG_OUT], ps2[:])
            # one DMA per image: SBUF [ho', (c w)] -> DRAM [c, ho', w]
            nc.sync.dma_start(
                out_ap[i].rearrange("(c hp) w -> hp c w", c=2),
                o_all[:].rearrange("hp (c w) -> hp c w", c=2))

    nc.compile()
    return nc


def _get_nc():
    if "nc" not in _CACHE:
        _CACHE["nc"] = _build()
    return _CACHE["nc"]


def kernel(**inputs) -> np.ndarray:
    from concourse.bass_utils import run_bass_kernel_spmd

    distance = np.ascontiguousarray(np.asarray(inputs["distance"], dtype=np.float32))
    assert distance.shape == (B, HW, M), distance.shape
    amat_t = _amat_t()

    nc = _get_nc()
    in_maps = []
    for c in range(N_CORES):
        shard = distance[c * BPC:(c + 1) * BPC].reshape(ROWS, M)
        in_maps.append({"distance": shard, "amat_t": amat_t})

    trace = bool(int(os.environ.get("KERNEL_TRACE", "0")))
    reps = int(os.environ.get("KERNEL_REPS", "1")) if trace else 1
    times = []
    res = None
    for _ in range(reps):
        try:
            res = run_bass_kernel_spmd(nc, in_maps,
                                       core_ids=list(range(N_CORES)),
                                       trace=trace)
        except ModuleNotFoundError:
            if not trace:
                raise
            trace = False
            res = run_bass_kernel_spmd(nc, in_maps,
                                       core_ids=list(range(N_CORES)),
                                       trace=False)
        if res.exec_time_ns is not None:
            times.append(res.exec_time_ns)
    if times:
        print(f"HW exec times: {times} -> min {min(times)} ns")
        _CACHE["exec_time_ns"] = min(times)
        _CACHE["results"] = res

    outs = [res.results[c]["out"] for c in range(N_CORES)]
    full = np.concatenate(outs, axis=0).reshape(B, 1, IMG_OUT, IMG_OUT)
    return full.astype(np.float32)


# revision 29
# speedup vs baseline: 1.0493x; 1.0493x over previous
"""AnomalyMapGenerator (retrieval kNN) Trainium2 kernel.

reference:  d = sqrt(distance[B, HW, M]); v = 3 smallest of d per row;
            w = softmax(-v); s = w0*v0 -> [B, 56, 56]
            -> bilinear resize to 224x224 -> gaussian blur (sigma=4, reflect).

Strategy (8 NeuronCores, data-parallel over batch, 2 images per core):
  - per core, rows r = b*3136 + hw (6272 rows of 4096 distances).
    Row->SBUF mapping r = 49*p + t: tile t holds rows {49p+t}, so the
    per-row scalar s lands in SBUF as [128, 49] in linear row order.
  - main loop (49 tiles of [128, 4096] f32, 2 MiB each):
      DMA load -> ScalarE negate -> VectorE max8 (top-8 of -d = 3 smallest of d,
      duplicate multiplicity preserved, matching lax.top_k).
  - tail: v = sqrt(-top3) (ScalarE), E = exp(-v) (ScalarE),
      s = v0*E0 / (E0+E1+E2) (VectorE), DMA s -> DRAM smap [6272].
  - post: resize+blur are one linear operator A = G_blur @ R_resize [224, 56];
      per image: out = A @ S @ A^T via two TensorE matmuls with
      amat_t = A^T [56, 224] (both stages use the same operand, no transposes).
"""
import os
import numpy as np

B, HW, M = 16, 3136, 4096
IMG_IN, IMG_OUT, SIGMA = 56, 224, 4.0
N_CORES = 8
BPC = B // N_CORES            # images per core
ROWS = BPC * HW               # 6272
P = 128
T = ROWS // P                 # 49 columns, row r = 49p + t
HALF = IMG_OUT // 2           # 112

# SDMA engine n of a transfer handles the n-th contiguous chunk of
# ceil(D/16) descriptors (measured).  Engine 15 is intermittently ~15-20%
# slower than the rest, so a few load units are issued as a [0:120)
# transfer (engines 0-14 only: 120 descriptors = 15 chunks of 8) plus a
# [120:128) transfer (8 descriptors -> engines 0-7), shifting ~14% of
# engine 15's bytes onto the others.  32 KiB descriptors (column pairs)
# are the per-engine throughput sweet spot; 64 KiB measured ~18% slower
# per byte.
SPLIT_UNITS = {5, 11, 17}

_CACHE = {}


def _resize_mat(in_size: int, out_size: int) -> np.ndarray:
    # jax.image.resize(method='bilinear') upsampling weight matrix [out, in]
    scale = out_size / in_size
    sample_f = (np.arange(out_size, dtype=np.float64) + 0.5) / scale - 0.5
    x = np.abs(sample_f[None, :] - np.arange(in_size, dtype=np.float64)[:, None])
    w = np.maximum(0.0, 1.0 - x)
    total = w.sum(axis=0, keepdims=True)
    w = np.where(np.abs(total) > 1e-8, w / total, 0.0)
    ob = (sample_f < -0.5) | (sample_f > in_size - 0.5)
    w[:, ob] = 0.0
    return w.T


def _gauss_mat(n: int, sigma: float) -> np.ndarray:
    # 1D gaussian conv with reflect padding as a matrix [n, n]
    ksize = 2 * int(4.0 * sigma + 0.5) + 1
    xs = np.arange(ksize, dtype=np.float64) - ksize // 2
    g = np.exp(-(xs * xs) / (2.0 * sigma * sigma))
    g = g / g.sum()
    pad = ksize // 2
    Gm = np.zeros((n, n), dtype=np.float64)
    for o in range(n):
        for k in range(ksize):
            idx = o - pad + k
            if idx < 0:
                idx = -idx
            elif idx > n - 1:
                idx = 2 * (n - 1) - idx
            Gm[o, idx] += g[k]
    return Gm


def _amat_t() -> np.ndarray:
    A = _gauss_mat(IMG_OUT, SIGMA) @ _resize_mat(IMG_IN, IMG_OUT)  # [224, 56]
    return np.ascontiguousarray(A.T.astype(np.float32))            # [56, 224]


def _build():
    from contextlib import ExitStack
    import concourse.bass as bass
    import concourse.tile as tile
    from concourse import bacc, mybir

    f32 = mybir.dt.float32
    AF = mybir.ActivationFunctionType

    nc = bacc.Bacc("TRN2", target_bir_lowering=False, debug=False,
                   enable_asserts=False)
    dist = nc.dram_tensor("distance", [ROWS, M], f32, kind="ExternalInput")
    amat = nc.dram_tensor("amat_t", [IMG_IN, IMG_OUT], f32, kind="ExternalInput")
    out = nc.dram_tensor("out", [BPC, IMG_OUT, IMG_OUT], f32, kind="ExternalOutput")
    smap = nc.dram_tensor("smap", [ROWS], f32)  # internal scratch

    distv = dist.ap().rearrange("(p t) m -> p t m", p=P)      # r = 49p + t
    smap_pt = smap.ap().rearrange("(p t) -> p t", p=P)
    smap_img = smap.ap().rearrange("(i h w) -> i h w", i=BPC, h=IMG_IN)
    out_ap = out.ap()

    with tile.TileContext(nc) as tc, ExitStack() as ctx:
        pool_in = ctx.enter_context(tc.tile_pool(name="in", bufs=3))
        pool_neg = ctx.enter_context(tc.tile_pool(name="neg", bufs=3))
        pool_keep = ctx.enter_context(tc.tile_pool(name="keep", bufs=1))
        pool_mm = ctx.enter_context(tc.tile_pool(name="mm", bufs=2))
        pool_ps = ctx.enter_context(
            tc.tile_pool(name="ps", bufs=4, space="PSUM"))

        # preload the sqrt activation table before ScalarE gets busy so the
        # tail pays no ACT_TABLE_LOAD (the in-loop negates are Copy, which is
        # in every table set, so sqrt stays resident)
        warm = pool_keep.tile([P, 8], f32)
        nc.vector.memset(warm[:], 1.0)
        nc.scalar.activation(warm[:], warm[:], AF.Sqrt)

        top8 = pool_keep.tile([P, 8 * T], f32)
        # paired loads: 2 adjacent t-columns are 32 KiB contiguous/partition.
        # negate per COLUMN so each max8 starts after half the pair's negate.
        units = [(2 * j, 2) for j in range(22)] + [(44, 1), (45, 1)]
        for u, (t0, w) in enumerate(units):
            tin = pool_in.tile([P, w * M], f32, tag="in")
            dst = tin[:].rearrange("p (k m) -> p k m", k=w)
            src = distv[:, t0:t0 + w, :]
            if u in SPLIT_UNITS:
                nc.sync.dma_start(dst[0:120], src[0:120])
                nc.sync.dma_start(dst[120:P], src[120:P])
            else:
                nc.sync.dma_start(dst, src)
            for k in range(w):
                t = t0 + k
                tneg = pool_neg.tile([P, M], f32, tag="neg")
                nc.scalar.mul(tneg[:], tin[:, k * M:(k + 1) * M], -1.0)
                nc.vector.max(top8[:, 8 * t:8 * t + 8], tneg[:])
        # stream-end taper: chunked columns so the post-DMA drain chain is
        # short; top-8 of a row = top-8 of the merged per-chunk top-8s
        for t, n_chunks in ((46, 2), (47, 2), (48, 4)):
            cw = M // n_chunks
            parts16 = pool_keep.tile([P, 8 * n_chunks], f32,
                                     tag=f"parts{n_chunks}")
            for h in range(n_chunks):
                tin = pool_in.tile([P, cw], f32, tag="in")
                nc.sync.dma_start(tin[:], distv[:, t, h * cw:(h + 1) * cw])
                tneg = pool_neg.tile([P, cw], f32, tag="neg")
                nc.scalar.mul(tneg[:], tin[:], -1.0)
                nc.vector.max(parts16[:, 8 * h:8 * h + 8], tneg[:])
            nc.vector.max(top8[:, 8 * t:8 * t + 8], parts16[:])

        amat_sb = pool_keep.tile([IMG_IN, IMG_OUT], f32)
        nc.sync.dma_start(amat_sb[:], amat.ap())

        # tail: softmin-weighted minimum per row
        #   s = v0 / (1 + e^{d1} + e^{d2}),  d_j = v0 - v_j  in [-1, 0]
        # One Sqrt activation (table warm); the exponentials use a cubic
        # Taylor poly on VectorE -- d is the gap between the closest and
        # 2nd/3rd-closest of 4096 distances, |d| <~ 0.05, so the cubic is
        # exact to ~1e-8 (worst-case |d|=1 error 0.035 still << tolerance).
        top8v = top8[:].rearrange("p (t e) -> p e t", e=8)
        vall = pool_keep.tile([P, 3 * T], f32)   # [v0 | v1 | v2] blocks
        nc.scalar.activation(
            vall[:].rearrange("p (e t) -> p e t", e=3),
            top8v[:, 0:3, :], AF.Sqrt, scale=-1.0)
        v0, v1, v2 = (vall[:, e * T:(e + 1) * T] for e in range(3))
        dd = pool_keep.tile([P, 2 * T], f32)
        nc.vector.tensor_sub(dd[:, 0:T], v0, v1)
        nc.vector.tensor_sub(dd[:, T:2 * T], v0, v2)
        # Horner: e^d ~ ((d/6 + 1/2)d + 1)d + 1
        ee = pool_keep.tile([P, 2 * T], f32)
        nc.vector.tensor_scalar(ee[:], dd[:], 1.0 / 6.0, 0.5,
                                mybir.AluOpType.mult, mybir.AluOpType.add)
        nc.vector.tensor_mul(ee[:], ee[:], dd[:])
        nc.vector.tensor_scalar_add(ee[:], ee[:], 1.0)
        nc.vector.tensor_mul(ee[:], ee[:], dd[:])
        denom = pool_keep.tile([P, T], f32)
        nc.vector.tensor_add(denom[:], ee[:, 0:T], ee[:, T:2 * T])
        # denom currently holds (e^{d1}-1) + (e^{d2}-1); add the 3 ones
        nc.vector.tensor_scalar_add(denom[:], denom[:], 3.0)
        rec = pool_keep.tile([P, T], f32)
        nc.vector.reciprocal(rec[:], denom[:])
        sval = pool_keep.tile([P, T], f32)
        nc.vector.tensor_mul(sval[:], v0, rec[:])
        # per-image halves: partitions [0,64) hold image 0's rows;
        # interleave each half's write with its S re-load
        s_tiles = []
        for i in range(BPC):
            h0, h1 = i * (P // 2), (i + 1) * (P // 2)
            nc.sync.dma_start(smap_pt[h0:h1, :], sval[h0:h1, :])
            s_i = pool_mm.tile([IMG_IN, IMG_IN], f32)
            nc.sync.dma_start(s_i[:], smap_img[i, :, :])
            s_tiles.append(s_i)

        # post: out_i = A @ S_i @ A^T
        for i in range(BPC):
            s_i = s_tiles[i]
            ps1 = pool_ps.tile([IMG_IN, IMG_OUT], f32)
            # ps1[w', n] = sum_h S[h, w'] * A[n, h]  ==  (A @ S)^T
            nc.tensor.matmul(ps1[:], s_i[:], amat_sb[:], start=True, stop=True)
            u1 = pool_mm.tile([IMG_IN, IMG_OUT], f32)
            (nc.vector.tensor_copy if i else nc.scalar.copy)(u1[:], ps1[:])
            o_all = pool_mm.tile([HALF, 2 * IMG_OUT], f32)
            for c in range(2):
                ps2 = pool_ps.tile([HALF, IMG_OUT], f32)
                # ps2[ho, w] = sum_w' (A@S)[ho, w'] * A[w, w']
                nc.tensor.matmul(ps2[:], u1[:, c * HALF:(c + 1) * HALF],
                                 amat_sb[:], start=True, stop=True)
                (nc.vector.tensor_copy if c else nc.scalar.copy)(
                    o_all[:, c * IMG_OUT:(c + 1) * IMG_OUT], ps2[:])
            # one DMA per image: SBUF [ho', (c w)] -> DRAM [c, ho', w]
            nc.sync.dma_start(
                out_ap[i].rearrange("(c hp) w -> hp c w", c=2),
                o_all[:].rearrange("hp (c w) -> hp c w", c=2))

    nc.compile()
    return nc


def _get_nc():
    if "nc" not in _CACHE:
        _CACHE["nc"] = _build()
    return _CACHE["nc"]


def kernel(**inputs) -> np.ndarray:
    from concourse.bass_utils import run_bass_kernel_spmd

    distance = np.ascontiguousarray(np.asarray(inputs["distance"], dtype=np.float32))
    assert distance.shape == (B, HW, M), distance.shape
    amat_t = _amat_t()

    nc = _get_nc()
    in_maps = []
    for c in range(N_CORES):
        shard = distance[c * BPC:(c + 1) * BPC].reshape(ROWS, M)
        in_maps.append({"distance": shard, "amat_t": amat_t})

    trace = bool(int(os.environ.get("KERNEL_TRACE", "0")))
    reps = int(os.environ.get("KERNEL_REPS", "1")) if trace else 1
    times = []
    res = None
    for _ in range(reps):
        try:
            res = run_bass_kernel_spmd(nc, in_maps,
                                       core_ids=list(range(N_CORES)),
                                       trace=trace)
        except ModuleNotFoundError:
            if not trace:
                raise
            trace = False
            res = run_bass_kernel_spmd(nc, in_maps,
                                       core_ids=list(range(N_CORES)),
                                       trace=False)
        if res.exec_time_ns is not None:
            times.append(res.exec_time_ns)
    if times:
        print(f"HW exec times: {times} -> min {min(times)} ns")
        _CACHE["exec_time_ns"] = min(times)
        _CACHE["results"] = res

    outs = [res.results[c]["out"] for c in range(N_CORES)]
    full = np.concatenate(outs, axis=0).reshape(B, 1, IMG_OUT, IMG_OUT)
    return full.astype(np.float32)
